# revision 17
# baseline (speedup 1.0000x reference)
"""Trainium2 Bass kernel for the DiffSSM block (v3).

Data-parallel over batch B=8 across 8 NeuronCores. Structure:

- Chunked low-rank SSM mix (exact): per-chunk diagonal Toeplitz blocks +
  64 fwd / 64 bwd geometric states with cross-chunk scans + prefix/suffix
  sums for the delta (Df/Db) terms. The fwd scan is interleaved into phase
  A; the bwd scan steps ride just-in-time inside phase B' which processes
  chunks in descending order, so no serial scan stall.
- Phase A and F loops are software-pipelined: iteration lt runs the
  matmuls + bn_stats + the Sqrt handoff for tile lt while finalizing tile
  lt-1 (reciprocal/normalize/stores), so the DVE queue never blocks on the
  ScalarE sqrt round-trip.
- LN stats (bn_stats) run directly on PSUM; trivial gains/biases
  (g=1, b=0 — true for this model) are skipped at build time via flags;
  the general paths emit K=1 matmuls / gpsimd passes instead.
- Per-lt DMA transposes of h_ln overlap phase A.
- Weight-stationary matmul orders amortize LDWEIGHTS; conv weights load
  pipelined into each conv phase's first output-tile pass (w2's first half
  preloads during conv1).
"""

import math

import numpy as np
import ml_dtypes

_BF16 = ml_dtypes.bfloat16

_L, _D, _B = 2048, 1024, 8

_cache = {}


def _build(L, D, n_cores, has_bi=False, has_b1=False, has_bo=False,
           has_g1=False, has_g2=False, has_b2=False):
    import concourse.bacc as bacc
    import concourse.bass as bass
    import concourse.tile as tile
    from concourse import mybir

    f32 = mybir.dt.float32
    bf16 = mybir.dt.bfloat16
    AF = mybir.ActivationFunctionType
    OP = mybir.AluOpType
    AX = mybir.AxisListType

    P = 128
    Q = 128                # SSM chunk size (== lt tile)
    KT = D // P
    LT = L // P
    NCk = LT
    NF = min(512, D)
    LC = L // NF

    nc = bacc.Bacc("TRN2", target_bir_lowering=False, debug=False,
                   num_devices=n_cores)

    x_res = nc.dram_tensor("x_res", (L, D), f32, kind="ExternalInput").ap()
    xTl = nc.dram_tensor("xTl", (LT, D, P), bf16, kind="ExternalInput").ap()
    Wi = nc.dram_tensor("Wi", (D, D), bf16, kind="ExternalInput").ap()
    w1T = nc.dram_tensor("w1T", (KT, P, 3, D), bf16, kind="ExternalInput").ap()
    w2T = nc.dram_tensor("w2T", (KT, P, 3, D), bf16, kind="ExternalInput").ap()
    Wo = nc.dram_tensor("Wo", (D, D), bf16, kind="ExternalInput").ap()
    lamS = nc.dram_tensor("lamS", (Q, P), bf16, kind="ExternalInput").ap()
    emat = nc.dram_tensor("emat", (P, Q), bf16, kind="ExternalInput").ap()
    tdiag = nc.dram_tensor("tdiag", (Q, Q), bf16, kind="ExternalInput").ap()
    lamQ = nc.dram_tensor("lamQ", (P, 1), f32, kind="ExternalInput").ap()
    bdfc = nc.dram_tensor("bdfc", (P, 1), f32, kind="ExternalInput").ap()
    bdbc = nc.dram_tensor("bdbc", (P, 1), f32, kind="ExternalInput").ap()
    nscT = nc.dram_tensor("nsc", (P, KT), f32, kind="ExternalInput").ap()
    bc1c = nc.dram_tensor("bc1c", (P, KT), f32, kind="ExternalInput").ap()
    bc2c = nc.dram_tensor("bc2c", (P, KT), f32, kind="ExternalInput").ap()
    if has_g1:
        g1r = nc.dram_tensor("g1r", (1, D), f32, kind="ExternalInput").ap()
    if has_g2:
        g2r = nc.dram_tensor("g2r", (1, D), f32, kind="ExternalInput").ap()
    if has_b2:
        b2r = nc.dram_tensor("b2r", (1, D), f32, kind="ExternalInput").ap()
    if has_bi:
        bir = nc.dram_tensor("bir", (1, D), bf16, kind="ExternalInput").ap()
    if has_bo:
        bor = nc.dram_tensor("bor", (1, D), bf16, kind="ExternalInput").ap()
    if has_b1:
        b1r = nc.dram_tensor("b1r", (1, D), bf16, kind="ExternalInput").ap()
        rrow = nc.dram_tensor("rrow", (1, L), bf16, kind="ExternalInput").ap()
        c0c = nc.dram_tensor("c0c", (P, KT), f32, kind="ExternalInput").ap()
        c2c = nc.dram_tensor("c2c", (P, KT), f32, kind="ExternalInput").ap()
    out = nc.dram_tensor("out", (L, D), f32, kind="ExternalOutput").ap()

    with tile.TileContext(nc) as tc:
        # ---------- pool stack (LIFO lifetimes), bottom first ----------
        const = tc.alloc_tile_pool(name="const", bufs=1)
        statp = tc.alloc_tile_pool(name="stat", bufs=4)
        dramp = tc.alloc_tile_pool(name="drams", bufs=1, space="DRAM")
        h2T_pool = tc.alloc_tile_pool(name="h2T", bufs=1)      # B'..F
        wo_pool = tc.alloc_tile_pool(name="wo", bufs=1)        # E..F
        hlnT_pool = tc.alloc_tile_pool(name="hlnT", bufs=1)    # A..E
        hln_pool = tc.alloc_tile_pool(name="hln", bufs=1)      # A..B'
        ssmc = tc.alloc_tile_pool(name="ssmc", bufs=1)         # A..B'
        dnsp = tc.alloc_tile_pool(name="dns", bufs=1)          # B'
        SM_pool = tc.alloc_tile_pool(name="SM", bufs=1)        # A..B'
        SC_pool = tc.alloc_tile_pool(name="SC", bufs=1)        # A..B'
        dl_pool = tc.alloc_tile_pool(name="dl", bufs=1)        # A..B'
        pa_pool = tc.alloc_tile_pool(name="pa", bufs=1)        # A only

        # ---- small constants ----
        ns_sb = const.tile([P, KT], f32)
        nc.scalar.dma_start(out=ns_sb[:], in_=nscT)
        bc1_sb = const.tile([P, KT], f32)
        nc.scalar.dma_start(out=bc1_sb[:], in_=bc1c)
        bc2_sb = const.tile([P, KT], f32)
        nc.scalar.dma_start(out=bc2_sb[:], in_=bc2c)
        lamQ_sb = const.tile([P, 1], f32)
        nc.scalar.dma_start(out=lamQ_sb[:], in_=lamQ)
        bdf_sb = const.tile([P, 1], f32)
        nc.scalar.dma_start(out=bdf_sb[:], in_=bdfc)
        bdb_sb = const.tile([P, 1], f32)
        nc.scalar.dma_start(out=bdb_sb[:], in_=bdbc)
        eps_sb = const.tile([P, 1], f32)
        nc.vector.memset(eps_sb[:], 1e-5)
        if has_bi or has_bo or has_b1:
            ones_sb = const.tile([1, P], bf16)
            nc.vector.memset(ones_sb[:], 1.0)
        if has_bi:
            bir_sb = const.tile([1, D], bf16)
            nc.scalar.dma_start(out=bir_sb[:], in_=bir)
        if has_bo:
            bor_sb = const.tile([1, D], bf16)
            nc.scalar.dma_start(out=bor_sb[:], in_=bor)
        if has_b1:
            b1r_sb = const.tile([1, D], bf16)
            nc.scalar.dma_start(out=b1r_sb[:], in_=b1r)
            rrow_sb = const.tile([1, L], bf16)
            nc.scalar.dma_start(out=rrow_sb[:], in_=rrow)
            c0_sb = const.tile([P, KT], f32)
            nc.scalar.dma_start(out=c0_sb[:], in_=c0c)
            c2_sb = const.tile([P, KT], f32)
            nc.scalar.dma_start(out=c2_sb[:], in_=c2c)

        lamS_sb = ssmc.tile([Q, P], bf16, tag="lamS", name="lamS")
        nc.scalar.dma_start(out=lamS_sb[:], in_=lamS)
        em_sb = ssmc.tile([P, Q], bf16, tag="em", name="em")
        nc.scalar.dma_start(out=em_sb[:], in_=emat)
        td_sb = ssmc.tile([Q, Q], bf16, tag="td", name="td")
        nc.scalar.dma_start(out=td_sb[:], in_=tdiag)

        h2T_sb = h2T_pool.tile([P, KT, L], bf16)
        wo_sb = wo_pool.tile([P, KT, D], bf16)
        hlnT_sb = hlnT_pool.tile([P, KT, L], bf16)
        hln_sb = hln_pool.tile([P, LT, D], bf16)
        SM_sb = SM_pool.tile([P, NCk, D], bf16)
        SC_sb = SC_pool.tile([P, NCk, D], bf16)
        DNS_sb = dnsp.tile([P, NCk, KT], f32)
        CS_sb = dl_pool.tile([P, NCk, KT], f32, tag="CS", name="CS")
        PS_sb = dl_pool.tile([P, NCk, KT], f32, tag="PS", name="PS")
        SUF_sb = dl_pool.tile([P, NCk, KT], f32, tag="SUF", name="SUF")
        TOT_sb = dl_pool.tile([P, KT], f32, tag="TOT", name="TOT")

        hln_dram = dramp.tile([L, D], bf16)
        hd_r = hln_dram[:].rearrange("(t p) d -> t p d", p=P)

        # phase A locals
        wi_sb = pa_pool.tile([P, KT, D], bf16)
        if has_g1:
            g1row = pa_pool.tile([1, D], f32, tag="g1row", name="g1row")
            nc.scalar.dma_start(out=g1row[:], in_=g1r)
            g1rep = pa_pool.tile([P, D], f32, tag="g1rep", name="g1rep")
            nc.gpsimd.partition_broadcast(g1rep[:], g1row[:])
        wi_r = Wi.rearrange("(kt p) d -> kt p d", p=P)
        xTl_r = xTl.rearrange("t (kt p) c -> t p kt c", p=P)

        psumA = tc.alloc_tile_pool(name="psumA", bufs=2, space="PSUM")
        psumSt = tc.alloc_tile_pool(name="psumSt", bufs=1, space="PSUM")

        nc.vector.memset(SC_sb[0:64, 0, :], 0.0)
        nc.vector.memset(SC_sb[64:P, NCk - 1, :], 0.0)
        nc.vector.memset(PS_sb[:, 0, :], 0.0)

        # ---- Phase A (software pipelined by one iteration) ----
        pend = {}

        def a_finalize(lt):
            ps0, ps1, mv, rstd = pend.pop(lt)
            nc.vector.reciprocal(out=rstd[:], in_=rstd[:])
            if has_g1:
                scr = scrA_tiles()
                nc.vector.tensor_scalar(out=scr[:, 0:NF], in0=ps0[:],
                                        scalar1=mv[:, 0:1], scalar2=rstd[:],
                                        op0=OP.subtract, op1=OP.mult)
                nc.vector.tensor_scalar(out=scr[:, NF:D], in0=ps1[:],
                                        scalar1=mv[:, 0:1], scalar2=rstd[:],
                                        op0=OP.subtract, op1=OP.mult)
                nc.gpsimd.tensor_mul(out=hln_sb[:, lt, :], in0=scr[:],
                                     in1=g1rep[:])
            else:
                # normalize on ScalarE (DVE is the A-phase bottleneck):
                # hln = Identity(ps*rstd + (-m*rstd))
                nb = statp.tile([P, 1], f32, tag="nb", name="nb")
                nc.scalar.mul(out=nb[:], in_=mv[:, 0:1], mul=rstd[:])
                nc.scalar.mul(out=nb[:], in_=nb[:], mul=-1.0)
                nc.scalar.activation(out=hln_sb[:, lt, 0:NF], in_=ps0[:],
                                     func=AF.Identity, scale=rstd[:],
                                     bias=nb[:])
                nc.scalar.activation(out=hln_sb[:, lt, NF:D], in_=ps1[:],
                                     func=AF.Identity, scale=rstd[:],
                                     bias=nb[:])
            nc.sync.dma_start(out=hd_r[lt], in_=hln_sb[:, lt, :])
            nc.scalar.dma_start_transpose(
                out=hlnT_sb[:, :, lt * P:(lt + 1) * P], in_=hd_r[lt])
            pstt = psumSt.tile([P, D], f32, tag="st", name="pstt", bufs=1)
            nc.tensor.matmul(pstt[:, 0:NF], lhsT=lamS_sb[:],
                             rhs=hln_sb[:, lt, 0:NF], start=True, stop=True)
            nc.tensor.matmul(pstt[:, NF:D], lhsT=lamS_sb[:],
                             rhs=hln_sb[:, lt, NF:D], start=True, stop=True)
            nc.scalar.activation(out=SM_sb[:, lt, :], in_=pstt[:],
                                 func=AF.Copy)
            # fwd scan step c=lt (reads SM[lt-1], finalized last iteration)
            if lt >= 1:
                nc.vector.scalar_tensor_tensor(
                    out=SC_sb[0:64, lt, :], in0=SC_sb[0:64, lt - 1, :],
                    scalar=lamQ_sb[0:64, :], in1=SM_sb[0:64, lt - 1, :],
                    op0=OP.mult, op1=OP.add)

        if has_g1:
            scrA_pool = tc.alloc_tile_pool(name="scrA", bufs=1)

            def scrA_tiles():
                return scrA_pool.tile([P, D], f32, tag="scr", name="scr",
                                      bufs=2)

        def a_lagged(lt):
            # cs-reduce + PS prefix, two iterations behind (transpose done)
            nc.vector.tensor_reduce(out=CS_sb[:, lt, :],
                                    in_=hlnT_sb[:, :, lt * P:(lt + 1) * P],
                                    axis=AX.X, op=OP.add)
            if lt + 1 < NCk:
                nc.gpsimd.tensor_add(out=PS_sb[:, lt + 1, :],
                                     in0=PS_sb[:, lt, :], in1=CS_sb[:, lt, :])

        # deep xTc prefetch: issue loads 3 iterations ahead of consumption
        xTc_tiles = {}

        def xTc_load(lt):
            if lt >= LT:
                return
            t = pa_pool.tile([P, KT, P], bf16, tag="xTc", name="xTc", bufs=4)
            nc.sync.dma_start(out=t[:], in_=xTl_r[lt])
            xTc_tiles[lt] = t

        # first-MM inputs land first: xTc(0), wi[0], then the rest interleaved
        xTc_load(0)
        nc.sync.dma_start(out=wi_sb[:, 0, :], in_=wi_r[0])
        xTc_load(1)
        xTc_load(2)
        for kt in range(1, KT):
            nc.sync.dma_start(out=wi_sb[:, kt, :], in_=wi_r[kt])
        for lt in range(LT):
            xTc = xTc_tiles.pop(lt)
            xTc_load(lt + 3)
            psA = psumA.tile([P, NF], f32, tag="a0", name="psA", bufs=3)
            psB = psumA.tile([P, NF], f32, tag="a1", name="psB", bufs=3)
            if has_bi:
                nc.tensor.matmul(psA[:], lhsT=ones_sb[:], rhs=bir_sb[:, 0:NF],
                                 start=True, stop=False)
                nc.tensor.matmul(psB[:], lhsT=ones_sb[:], rhs=bir_sb[:, NF:D],
                                 start=True, stop=False)
            for kt in range(KT):
                st = (kt == 0) and not has_bi
                nc.tensor.matmul(psA[:], lhsT=xTc[:, kt, :],
                                 rhs=wi_sb[:, kt, 0:NF],
                                 start=st, stop=(kt == KT - 1))
                nc.tensor.matmul(psB[:], lhsT=xTc[:, kt, :],
                                 rhs=wi_sb[:, kt, NF:D],
                                 start=st, stop=(kt == KT - 1))
            stats = statp.tile([P, 2, 6], f32, tag="stats", name="stats")
            nc.vector.bn_stats(out=stats[:, 0, :], in_=psA[:])
            nc.vector.bn_stats(out=stats[:, 1, :], in_=psB[:])
            mv = statp.tile([P, 2], f32, tag="mv", name="mv")
            nc.vector.bn_aggr(out=mv[:], in_=stats[:])
            rstd = statp.tile([P, 1], f32, tag="rstd", name="rstd")
            nc.scalar.activation(out=rstd[:], in_=mv[:, 1:2], func=AF.Sqrt,
                                 bias=eps_sb[:], scale=1.0)
            pend[lt] = (psA, psB, mv, rstd)
            if lt >= 1:
                a_finalize(lt - 1)
            if lt >= 2:
                a_lagged(lt - 2)
        a_finalize(LT - 1)
        a_lagged(LT - 2)
        a_lagged(LT - 1)
        nc.vector.tensor_add(out=TOT_sb[:], in0=PS_sb[:, NCk - 1, :],
                             in1=CS_sb[:, NCk - 1, :])
        if has_g1:
            scrA_pool.release()
        pa_pool.release()
        psumSt.release()
        psumA.release()

        psumB = tc.alloc_tile_pool(name="psumB", bufs=8, space="PSUM")

        # ---- Phase B': chunks processed 15..0; bwd-scan + delta JIT ----
        for c in range(NCk - 1, -1, -1):
            if c < NCk - 1:
                nc.vector.scalar_tensor_tensor(
                    out=SC_sb[64:P, c, :], in0=SC_sb[64:P, c + 1, :],
                    scalar=lamQ_sb[64:P, :], in1=SM_sb[64:P, c + 1, :],
                    op0=OP.mult, op1=OP.add)
            # delta for this chunk: DNS = ns*(bdf*PS[c] + bdb*(TOT-PS[c]-CS[c]))
            nc.vector.tensor_sub(out=SUF_sb[:, c, :], in0=TOT_sb[:],
                                 in1=PS_sb[:, c, :])
            nc.vector.tensor_sub(out=SUF_sb[:, c, :], in0=SUF_sb[:, c, :],
                                 in1=CS_sb[:, c, :])
            nc.vector.tensor_scalar_mul(out=DNS_sb[:, c, :],
                                        in0=PS_sb[:, c, :], scalar1=bdf_sb[:])
            nc.vector.scalar_tensor_tensor(
                out=DNS_sb[:, c, :], in0=SUF_sb[:, c, :], scalar=bdb_sb[:],
                in1=DNS_sb[:, c, :], op0=OP.mult, op1=OP.add)
            nc.vector.tensor_mul(out=DNS_sb[:, c, :], in0=DNS_sb[:, c, :],
                                 in1=ns_sb[:])
            for dt in range(KT):
                psb = psumB.tile([P, Q], f32, tag="b", name="psb")
                nc.tensor.matmul(psb[:], lhsT=hln_sb[:, c, dt * P:(dt + 1) * P],
                                 rhs=td_sb[:], start=True, stop=False)
                if has_b1:
                    nc.tensor.matmul(psb[:],
                                     lhsT=b1r_sb[:, dt * P:(dt + 1) * P],
                                     rhs=rrow_sb[:, c * Q:(c + 1) * Q],
                                     start=False, stop=False)
                nc.tensor.matmul(psb[:], lhsT=SC_sb[:, c, dt * P:(dt + 1) * P],
                                 rhs=em_sb[:], start=False, stop=True)
                # eviction on ScalarE: h2T = Identity(ps*ns + DNS)
                nc.scalar.activation(
                    out=h2T_sb[:, dt, c * Q:(c + 1) * Q], in_=psb[:],
                    func=AF.Identity, scale=ns_sb[:, dt:dt + 1],
                    bias=DNS_sb[:, c, dt:dt + 1])
        psumB.release()
        dl_pool.release()
        SC_pool.release()
        SM_pool.release()
        dnsp.release()
        ssmc.release()
        hln_pool.release()

        # ---- conv pools: co + w2-first-half + w1 (w1 dies first) ----
        co_pool = tc.alloc_tile_pool(name="co", bufs=1)
        co_sb = co_pool.tile([P, KT, L], bf16)
        w2a_pool = tc.alloc_tile_pool(name="w2a", bufs=1)
        w2a_sb = w2a_pool.tile([P, KT // 2, 3, D], bf16)
        w1_pool = tc.alloc_tile_pool(name="w1", bufs=1)
        w1_sb = w1_pool.tile([P, KT, 3, D], bf16)
        for it in range(KT):
            nc.scalar.dma_start(out=w1_sb[:, it, :, :], in_=w1T[it])
        for it in range(KT // 2):
            nc.scalar.dma_start(out=w2a_sb[:, it, :, :], in_=w2T[it])
        psumD = tc.alloc_tile_pool(name="psumD", bufs=8, space="PSUM")

        def conv_phase(wsel, src_sb, evict):
            for ot in range(KT):
                pss = [psumD.tile([P, NF], f32, tag=f"d{ot % 2}{lc}",
                                  name=f"ps{lc}", bufs=1) for lc in range(LC)]
                for it in range(KT):
                    wt, wit = wsel(it)
                    js = (1, 0, 2) if it == 0 else (0, 1, 2)
                    for j in js:
                        last = (it == KT - 1 and j == js[-1])
                        for lc in range(LC):
                            o0 = 1 if (j == 0 and lc == 0) else 0
                            o1 = NF - 1 if (j == 2 and lc == LC - 1) else NF
                            base = lc * NF + j - 1
                            nc.tensor.matmul(
                                pss[lc][:, o0:o1],
                                lhsT=wt[:, wit, j, ot * P:(ot + 1) * P],
                                rhs=src_sb[:, it, base + o0:base + o1],
                                start=(it == 0 and j == 1), stop=last)
                for lc in range(LC):
                    evict(ot, lc, pss[lc])

        def evict1(ot, lc, ps):
            if has_b1 and lc == 0:
                nc.vector.tensor_sub(out=ps[:, 0:1], in0=ps[:, 0:1],
                                     in1=c0_sb[:, ot:ot + 1])
            if has_b1 and lc == LC - 1:
                nc.vector.tensor_sub(out=ps[:, NF - 1:NF],
                                     in0=ps[:, NF - 1:NF],
                                     in1=c2_sb[:, ot:ot + 1])
            nc.scalar.activation(out=co_sb[:, ot, lc * NF:(lc + 1) * NF],
                                 in_=ps[:], func=AF.Silu,
                                 bias=bc1_sb[:, ot:ot + 1], scale=1.0)

        conv_phase(lambda it: (w1_sb, it), hlnT_sb, evict1)
        w1_pool.release()

        # second half of w2 (loads during conv2's first output tiles)
        w2b_pool = tc.alloc_tile_pool(name="w2b", bufs=1)
        w2b_sb = w2b_pool.tile([P, KT - KT // 2, 3, D], bf16)
        for it in range(KT - KT // 2):
            nc.scalar.dma_start(out=w2b_sb[:, it, :, :], in_=w2T[KT // 2 + it])
        wo_r = Wo.rearrange("(dt p) e -> dt p e", p=P)
        for dt in range(KT):
            nc.scalar.dma_start(out=wo_sb[:, dt, :], in_=wo_r[dt])

        def evict2(ot, lc, ps):
            nc.vector.scalar_tensor_tensor(
                out=h2T_sb[:, ot, lc * NF:(lc + 1) * NF], in0=ps[:],
                scalar=bc2_sb[:, ot:ot + 1],
                in1=h2T_sb[:, ot, lc * NF:(lc + 1) * NF],
                op0=OP.add, op1=OP.add)

        def w2sel(it):
            if it < KT // 2:
                return (w2a_sb, it)
            return (w2b_sb, it - KT // 2)

        conv_phase(w2sel, co_sb, evict2)
        psumD.release()
        w2b_pool.release()
        w2a_pool.release()
        co_pool.release()
        hlnT_pool.release()

        # ---- Phase F: proj-out + LN2 + residual (software pipelined) ----
        scrF = tc.alloc_tile_pool(name="scrF", bufs=1)
        if has_g2:
            g2row = scrF.tile([1, D], f32, tag="g2row", name="g2row")
            nc.scalar.dma_start(out=g2row[:], in_=g2r)
            g2rep = scrF.tile([P, D], f32, tag="g2rep", name="g2rep")
            nc.gpsimd.partition_broadcast(g2rep[:], g2row[:])
        if has_b2:
            b2row = scrF.tile([1, D], f32, tag="b2row", name="b2row")
            nc.scalar.dma_start(out=b2row[:], in_=b2r)
            b2rep = scrF.tile([P, D], f32, tag="b2rep", name="b2rep")
            nc.gpsimd.partition_broadcast(b2rep[:], b2row[:])
        hbufp = tc.alloc_tile_pool(name="hbuf", bufs=1)
        psumF = tc.alloc_tile_pool(name="psumF", bufs=2, space="PSUM")
        x_r = x_res.rearrange("(t p) d -> t p d", p=P)
        out_r = out.rearrange("(t p) d -> t p d", p=P)
        pendF = {}

        def f_finalize(lt):
            ps0, ps1, mv, rstd, xin = pendF.pop(lt)
            nc.vector.reciprocal(out=rstd[:], in_=rstd[:])
            yn = scrF.tile([P, D], f32, tag="yn", name="yn", bufs=2)
            nb = statp.tile([P, 1], f32, tag="nb", name="nb")
            nc.scalar.mul(out=nb[:], in_=mv[:, 0:1], mul=rstd[:])
            nc.scalar.mul(out=nb[:], in_=nb[:], mul=-1.0)
            nc.scalar.activation(out=yn[:, 0:NF], in_=ps0[:],
                                 func=AF.Identity, scale=rstd[:], bias=nb[:])
            nc.scalar.activation(out=yn[:, NF:D], in_=ps1[:],
                                 func=AF.Identity, scale=rstd[:], bias=nb[:])
            out_t = hbufp.tile([P, D], f32, tag="out_t", name="out_t", bufs=2)
            if has_g2:
                t1 = hbufp.tile([P, D], f32, tag="t1", name="t1", bufs=2)
                nc.gpsimd.tensor_mul(out=t1[:], in0=yn[:], in1=g2rep[:])
                nc.vector.tensor_add(out=out_t[:], in0=t1[:], in1=xin[:])
            else:
                nc.gpsimd.tensor_add(out=out_t[:], in0=yn[:], in1=xin[:])
            nc.scalar.dma_start(out=out_r[lt], in_=out_t[:])

        for lt in range(LT):
            x_t = hbufp.tile([P, D], f32, tag="x_t", name="x_t", bufs=3)
            nc.scalar.dma_start(out=x_t[:], in_=x_r[lt])
            if has_b2:
                xb2 = hbufp.tile([P, D], f32, tag="xb2", name="xb2", bufs=2)
                nc.gpsimd.tensor_add(out=xb2[:], in0=x_t[:], in1=b2rep[:])
                xin = xb2
            else:
                xin = x_t
            psA = psumF.tile([P, NF], f32, tag="f0", name="psA", bufs=3)
            psB = psumF.tile([P, NF], f32, tag="f1", name="psB", bufs=3)
            if has_bo:
                nc.tensor.matmul(psA[:], lhsT=ones_sb[:], rhs=bor_sb[:, 0:NF],
                                 start=True, stop=False)
                nc.tensor.matmul(psB[:], lhsT=ones_sb[:], rhs=bor_sb[:, NF:D],
                                 start=True, stop=False)
            for dt in range(KT):
                st = (dt == 0) and not has_bo
                nc.tensor.matmul(psA[:], lhsT=h2T_sb[:, dt, lt * P:(lt + 1) * P],
                                 rhs=wo_sb[:, dt, 0:NF],
                                 start=st, stop=(dt == KT - 1))
                nc.tensor.matmul(psB[:], lhsT=h2T_sb[:, dt, lt * P:(lt + 1) * P],
                                 rhs=wo_sb[:, dt, NF:D],
                                 start=st, stop=(dt == KT - 1))
            stats = statp.tile([P, 2, 6], f32, tag="stats", name="stats")
            nc.vector.bn_stats(out=stats[:, 0, :], in_=psA[:])
            nc.vector.bn_stats(out=stats[:, 1, :], in_=psB[:])
            mv = statp.tile([P, 2], f32, tag="mv", name="mv")
            nc.vector.bn_aggr(out=mv[:], in_=stats[:])
            rstd = statp.tile([P, 1], f32, tag="rstd", name="rstd")
            nc.scalar.activation(out=rstd[:], in_=mv[:, 1:2], func=AF.Sqrt,
                                 bias=eps_sb[:], scale=1.0)
            pendF[lt] = (psA, psB, mv, rstd, xin)
            if lt >= 1:
                f_finalize(lt - 1)
        f_finalize(LT - 1)
        psumF.release()
        hbufp.release()
        scrF.release()
        wo_pool.release()
        h2T_pool.release()
        dramp.release()
        statp.release()
        const.release()

    nc.compile()
    return nc


def _bf(a):
    return np.ascontiguousarray(np.asarray(a, np.float32)).astype(_BF16)


def _flags(inputs):
    f32 = np.float32
    return (bool(np.any(np.asarray(inputs["bi"], f32))),
            bool(np.any(np.asarray(inputs["b1"], f32))),
            bool(np.any(np.asarray(inputs["bo"], f32))),
            bool(np.any(np.asarray(inputs["g1"], f32) != 1.0)),
            bool(np.any(np.asarray(inputs["g2"], f32) != 1.0)),
            bool(np.any(np.asarray(inputs["b2"], f32))))


def _prep_maps(inputs, L, D, n_cores):
    P = 128
    Q = 128
    KT = D // P
    LT = L // P
    f32 = np.float32
    x = np.asarray(inputs["x"], f32)
    t = np.asarray(inputs["t"], f32)
    beta1 = f32(np.asarray(inputs["beta1"], f32)[0])
    beta2 = f32(np.asarray(inputs["beta2"], f32)[0])

    af = np.diagonal(np.asarray(inputs["Af"], f32)).astype(f32)
    ab = np.diagonal(np.asarray(inputs["Ab"], f32)).astype(f32)
    wfv = (np.asarray(inputs["Bf"], f32)[:, 0]
           * np.asarray(inputs["Cf"], f32)[0]).astype(f32)
    wbv = (np.asarray(inputs["Bb"], f32)[:, 0]
           * np.asarray(inputs["Cb"], f32)[0]).astype(f32)
    Df = f32(np.asarray(inputs["Df"], f32)[0])
    Db = f32(np.asarray(inputs["Db"], f32)[0])

    lar = np.arange(L, dtype=f32)[:, None]
    kf = np.exp(lar * af[None, :]) @ wfv + Df
    kb = np.exp(lar * ab[None, :]) @ wbv + Db

    s = np.arange(Q, dtype=f32)
    LamS = np.concatenate([np.exp(af[None, :] * (Q - 1 - s)[:, None]),
                           np.exp(ab[None, :] * s[:, None])], axis=1)
    lamQv = np.concatenate([np.exp(af * Q), np.exp(ab * Q)]).astype(f32)
    tau = np.arange(Q, dtype=f32)
    Emat = np.concatenate([
        beta1 * wfv[:, None] * np.exp(af[:, None] * (tau + 1)[None, :]),
        beta2 * wbv[:, None] * np.exp(ab[:, None] * (Q - tau)[None, :])],
        axis=0)
    dd = tau[None, :] - s[:, None]
    idx = np.clip(dd, 0, None).astype(np.int64)
    idxn = np.clip(-dd, 0, None).astype(np.int64)
    Tdiag = (np.where(dd >= 0, beta1 * kf[idx], 0.0)
             + np.where(dd <= 0, beta2 * kb[idxn], 0.0)).astype(f32)

    half = D // 2
    freqs = np.exp(np.arange(half, dtype=f32)
                   * (-math.log(10000.0) / (half - 1)))
    ang = t[:, None] * freqs[None, :]
    emb = np.concatenate([np.sin(ang), np.cos(ang)], axis=1).astype(f32)
    ns = (1.0 / (1.0 + np.exp(-emb))).astype(f32)

    w1 = np.asarray(inputs["w1"], f32)
    w2 = np.asarray(inputs["w2"], f32)
    b1v = np.asarray(inputs["b1"], f32)
    w1T = np.ascontiguousarray(np.transpose(w1, (1, 2, 0))).reshape(
        KT, P, 3, D).astype(_BF16)
    w2T = np.ascontiguousarray(np.transpose(w2, (1, 2, 0))).reshape(
        KT, P, 3, D).astype(_BF16)

    def col(v):
        return np.ascontiguousarray(np.asarray(v, f32).reshape(KT, P).T)

    def row(v, dt=f32):
        return np.ascontiguousarray(np.asarray(v, f32).reshape(1, -1)
                                    ).astype(dt)

    has_bi, has_b1, has_bo, has_g1, has_g2, has_b2 = _flags(inputs)

    C1 = sum(w1[:, :, j] @ b1v for j in range(3)) + np.asarray(
        inputs["bc1"], f32)
    shared = {
        "Wi": _bf(inputs["Wi"]), "Wo": _bf(inputs["Wo"]),
        "w1T": w1T, "w2T": w2T,
        "lamS": LamS.astype(_BF16), "emat": Emat.astype(_BF16),
        "tdiag": Tdiag.astype(_BF16),
        "lamQ": np.ascontiguousarray(lamQv.reshape(P, 1)),
        "bdfc": np.full((P, 1), beta1 * Df, f32),
        "bdbc": np.full((P, 1), beta2 * Db, f32),
        "bc1c": col(C1), "bc2c": col(inputs["bc2"]),
    }
    if has_g1:
        shared["g1r"] = row(inputs["g1"])
    if has_g2:
        shared["g2r"] = row(inputs["g2"])
    if has_b2:
        shared["b2r"] = row(inputs["b2"])
    if has_bi:
        shared["bir"] = row(inputs["bi"], _BF16)
    if has_bo:
        shared["bor"] = row(inputs["bo"], _BF16)
    if has_b1:
        shared["b1r"] = row(b1v, _BF16)
        ckf = np.cumsum(kf)
        ckb = np.cumsum(kb)
        tt = np.arange(L)
        rr = beta1 * ckf[tt] + beta2 * ckb[L - 1 - tt]
        shared["rrow"] = row(rr, _BF16)
        shared["c0c"] = col(w1[:, :, 0] @ b1v)
        shared["c2c"] = col(w1[:, :, 2] @ b1v)

    in_maps = []
    for b in range(n_cores):
        xb = np.ascontiguousarray(x[b])
        m = dict(shared)
        m["x_res"] = xb
        xbT = xb.T.astype(_BF16)                      # (D, L)
        m["xTl"] = np.ascontiguousarray(
            xbT.reshape(1024, LT, P).transpose(1, 0, 2))
        m["nsc"] = np.ascontiguousarray(ns[b].reshape(KT, P).T)
        in_maps.append(m)
    return in_maps


def get_nc(L=_L, D=_D, n_cores=_B, flags=(False,) * 6):
    key = (L, D, n_cores) + tuple(flags)
    if key not in _cache:
        _cache[key] = _build(L, D, n_cores, *flags)
    return _cache[key]


def kernel(**inputs):
    from concourse.bass_utils import run_bass_kernel_spmd

    L, D, B = _L, _D, _B
    flags = _flags(inputs)
    nc = get_nc(L, D, B, flags)
    in_maps = _prep_maps(inputs, L, D, B)
    res = run_bass_kernel_spmd(nc, in_maps, core_ids=list(range(B)))
    return np.stack([res.results[c]["out"] for c in range(B)]).astype(
        np.float32)
